# revision 13
# baseline (speedup 1.0000x reference)
"""DANetHead (dual attention head) Trainium2 kernel.

Strategy (8 NeuronCores): 2-way data parallel over batch B=2 (core groups
[0-3], [4-7]) x 4-way model parallel within each batch group:
  - Stage-1 3x3 convs (2048->512): split over output channels (4 x 128).
  - Attention + stage-2: split over pixels (4 x 15 rows of the 60x60 image);
    feature maps exchanged via AllGather, CAM gram matrix via AllReduce.

Matmuls run in bf16 (f32 PSUM accumulation) except the attention/CAM logits
which use f32 / hi-lo bf16 splitting to keep softmax inputs accurate.
"""

from contextlib import ExitStack

import numpy as np
import ml_dtypes

import concourse.bass as bass
import concourse.tile as tile
import concourse.mybir as mybir
from concourse.bass import ds

dt = mybir.dt
F32 = dt.float32
BF16 = dt.bfloat16
AF = mybir.ActivationFunctionType
AX = mybir.AxisListType
ALU = mybir.AluOpType

P = 128
H = 60
HP = 62
NPIX = 3600          # 60*60
NPAD = 3720          # 60 zero + 3600 + 60 zero (padded gathered feature)
MP = 3712            # 29*128, padded key/value pixel count
MCH = 29             # m chunks
WIN = 1020           # 17 rows * 60 query window
WINP = 1024          # padded window
CIN = 2048
CICN = 16            # input channel chunks (stage 1)
CI = 512
CIC = 4              # 512 / 128
CQ = 64
CO = 40
CSH = 25.0           # softmax shift constant (max logit ~24.8)
GROUPS = [[0, 1, 2, 3], [4, 5, 6, 7]]
EPS = 1e-5

bf = ml_dtypes.bfloat16


# ---------------------------------------------------------------- builder ---

def build_nc(split=True):
    nc = bass.Bass(num_devices=8)

    # ---- inputs (per-core contents differ; shapes identical) ----
    XPAD = nc.dram_tensor("XPAD", [CICN, P, HP * HP], BF16, kind="ExternalInput")
    W0S = nc.dram_tensor("W0S", [9, CICN, P, P], BF16, kind="ExternalInput")
    W0C = nc.dram_tensor("W0C", [9, CICN, P, P], BF16, kind="ExternalInput")
    BN0S = nc.dram_tensor("BN0S", [P, 2], F32, kind="ExternalInput")
    BN0C = nc.dram_tensor("BN0C", [P, 2], F32, kind="ExternalInput")
    WQT = nc.dram_tensor("WQT", [CIC, P, CQ], F32, kind="ExternalInput")
    WKT = nc.dram_tensor("WKT", [CIC, P, CQ], F32, kind="ExternalInput")
    BQ = nc.dram_tensor("BQ", [CQ, 1], F32, kind="ExternalInput")
    BK = nc.dram_tensor("BK", [CQ, 1], F32, kind="ExternalInput")
    WVT = nc.dram_tensor("WVT", [CIC, P, CI], BF16, kind="ExternalInput")
    BV = nc.dram_tensor("BV", [P, CIC], F32, kind="ExternalInput")
    DKA = nc.dram_tensor("DKA", [2, MP], F32, kind="ExternalInput")
    DQA = nc.dram_tensor("DQA", [2, WINP], F32, kind="ExternalInput")
    QMASK = nc.dram_tensor("QMASK", [1, WINP], F32, kind="ExternalInput")
    GSA = nc.dram_tensor("GSA", [1, P], F32, kind="ExternalInput")
    GSC = nc.dram_tensor("GSC", [P, 1], F32, kind="ExternalInput")
    W1S = nc.dram_tensor("W1S", [9, CIC, P, CI], BF16, kind="ExternalInput")
    W1C = nc.dram_tensor("W1C", [9, CIC, P, CI], BF16, kind="ExternalInput")
    BN1S = nc.dram_tensor("BN1S", [P, 2, CIC], F32, kind="ExternalInput")
    BN1C = nc.dram_tensor("BN1C", [P, 2, CIC], F32, kind="ExternalInput")
    W6T = nc.dram_tensor("W6T", [CIC, P, CO], BF16, kind="ExternalInput")
    W7T = nc.dram_tensor("W7T", [CIC, P, CO], BF16, kind="ExternalInput")
    W8T = nc.dram_tensor("W8T", [CIC, P, CO], BF16, kind="ExternalInput")
    B6 = nc.dram_tensor("B6", [CO, 1], F32, kind="ExternalInput")
    B7 = nc.dram_tensor("B7", [CO, 1], F32, kind="ExternalInput")
    B8 = nc.dram_tensor("B8", [CO, 1], F32, kind="ExternalInput")
    ZPAD = nc.dram_tensor("ZPAD", [CIC, P, H], F32, kind="ExternalInput")
    OUT = nc.dram_tensor("OUT", [3, CO, 900], F32, kind="ExternalOutput")

    with tile.TileContext(nc) as tc, ExitStack() as ctx:
        dram = ctx.enter_context(tc.tile_pool(name="dram", bufs=1, space="DRAM"))

        # window start within the padded gathered features: 900 * (core % 4)
        woff = (nc.sync.partition_id() % 4) * 900

        f1in = dram.tile([P, NPIX], F32, name="f1in")
        f2in = dram.tile([P, NPIX], F32, name="f2in")
        f1g = dram.tile([CIC, P, NPAD], F32, name="f1g")
        f2g = dram.tile([CIC, P, NPAD], F32, name="f2g")
        f1gc = dram.tile([CIC, P, NPIX], F32, name="f1gc")
        f2gc = dram.tile([CIC, P, NPIX], F32, name="f2gc")
        cen_in = dram.tile([CIC, P, CI], F32, name="cen_in")
        cen_out = dram.tile([CIC, P, CI], F32, name="cen_out")

        for fg_ in (f1g, f2g):
            nc.sync.dma_start(fg_[:, :, 0:H], ZPAD[:])
            nc.sync.dma_start(fg_[:, :, NPAD - H: NPAD], ZPAD[:])

        # =========================== stage 1: 3x3 convs 2048 -> 128 ==========
        with ExitStack() as c1:
            sb1 = c1.enter_context(tc.tile_pool(name="sb1", bufs=1))
            wp1 = c1.enter_context(tc.tile_pool(name="wp1", bufs=4))
            pp1 = c1.enter_context(tc.tile_pool(name="pp1", bufs=8, space="PSUM"))

            xpad = sb1.tile([P, CICN, HP * HP], BF16, name="xpad")
            for cic in range(CICN):
                nc.sync.dma_start(xpad[:, cic, :], XPAD[cic])

            bns = sb1.tile([P, 2], F32, name="bns")
            bnc = sb1.tile([P, 2], F32, name="bnc")
            nc.sync.dma_start(bns[:], BN0S[:])
            nc.sync.dma_start(bnc[:], BN0C[:])

            feats = {}
            for name, wsrc, bnt, fin, fg, fgc in (
                ("f1", W0S, bns, f1in, f1g, f1gc),
                ("f2", W0C, bnc, f2in, f2g, f2gc),
            ):
                feat = sb1.tile([P, NPIX], F32, name=f"feat_{name}")
                feats[name] = feat
                pts = [
                    pp1.tile([P, 480], F32, name="s1p", tag="s1p") for _ in range(8)
                ]
                for cic in range(CICN):
                    for off in range(9):
                        ky, kx = off // 3, off % 3
                        wt = wp1.tile([P, P], BF16, name="w0t", tag="w0t")
                        nc.gpsimd.dma_start(wt[:], wsrc[off, cic])
                        xv = xpad[:, cic, :].rearrange("p (r c) -> p r c", c=HP)
                        start = cic == 0 and off == 0
                        stop = cic == CICN - 1 and off == 8
                        for t in range(8):
                            rows = 8 if t < 7 else 4
                            rhs = xv[:, ky + 8 * t: ky + 8 * t + rows, kx: kx + H]
                            nc.tensor.matmul(
                                pts[t][:, : rows * H], wt[:], rhs,
                                start=start, stop=stop,
                            )
                for t in range(8):
                    rows = 8 if t < 7 else 4
                    nc.scalar.activation(
                        feat[:, t * 480: t * 480 + rows * H],
                        pts[t][:, : rows * H],
                        AF.Relu, bias=bnt[:, 1:2], scale=bnt[:, 0:1],
                    )
                nc.sync.dma_start(fin[:], feat[:])
                nc.gpsimd.collective_compute(
                    "AllGather", ALU.bypass,
                    replica_groups=GROUPS,
                    ins=[fin.opt()],
                    outs=[fgc.opt()],
                )
                for cic in range(CIC):
                    nc.sync.dma_start(fg[cic, :, H: H + NPIX], fgc[cic])

        # ====================== phase 2: windows, k, q, v ====================
        pers = ctx.enter_context(tc.tile_pool(name="pers", bufs=1))
        mid = ctx.enter_context(tc.tile_pool(name="mid", bufs=1))
        f1win = [pers.tile([P, WINP], F32, name=f"f1win{i}") for i in range(CIC)]
        f2win = [pers.tile([P, WINP], F32, name=f"f2win{i}") for i in range(CIC)]
        for i in range(CIC):
            nc.any.memset(f1win[i][:], 0.0)
            nc.any.memset(f2win[i][:], 0.0)
            nc.sync.dma_start(f1win[i][:, 0:WIN], f1g[i, :, ds(woff, WIN)])
            nc.sync.dma_start(f2win[i][:, 0:WIN], f2g[i, :, ds(woff, WIN)])

        wqt = [pers.tile([P, CQ], F32, name=f"wqt{i}") for i in range(CIC)]
        wkt = [pers.tile([P, CQ], F32, name=f"wkt{i}") for i in range(CIC)]
        wvt = [pers.tile([P, CI], BF16, name=f"wvt{i}") for i in range(CIC)]
        for i in range(CIC):
            nc.sync.dma_start(wqt[i][:], WQT[i])
            nc.sync.dma_start(wkt[i][:], WKT[i])
            nc.sync.dma_start(wvt[i][:], WVT[i])
        bq = pers.tile([CQ, 1], F32, name="bq", padded_shape=[P, 1])
        bk = pers.tile([CQ, 1], F32, name="bk", padded_shape=[P, 1])
        bv = pers.tile([P, CIC], F32, name="bv")
        nc.sync.dma_start(bq[:], BQ[:])
        nc.sync.dma_start(bk[:], BK[:])
        nc.sync.dma_start(bv[:], BV[:])
        gsa = pers.tile([1, P], F32, name="gsa", padded_shape=[P, P])
        gsc = pers.tile([P, 1], F32, name="gsc")
        qmask = pers.tile([1, WINP], F32, name="qmask", padded_shape=[P, WINP])
        nc.sync.dma_start(gsa[:], GSA[:])
        nc.sync.dma_start(gsc[:], GSC[:])
        nc.sync.dma_start(qmask[:], QMASK[:])

        ka = mid.tile([P, MP], F32, name="ka")
        qa = mid.tile([P, WINP], F32, name="qa")
        nc.any.memset(ka[:], 0.0)
        nc.any.memset(qa[:], 0.0)
        nc.sync.dma_start(ka[64:66, :], DKA[:])
        nc.sync.dma_start(qa[64:66, :], DQA[:])

        vt = [pers.tile([P, MCH, P], BF16, name=f"vt{i}") for i in range(CIC)]

        with ExitStack() as c2:
            sb2 = c2.enter_context(tc.tile_pool(name="sb2", bufs=1))
            rp2 = c2.enter_context(tc.tile_pool(name="rp2", bufs=1))
            pk = c2.enter_context(tc.tile_pool(name="pk", bufs=8, space="PSUM"))

            vsp = c2.enter_context(tc.tile_pool(name="vsp", bufs=2))
            f1h = [sb2.tile([P, NPIX], BF16, name=f"f1h{i}") for i in range(CIC)]
            kps = [pk.tile([CQ, 450], F32, name="kp", tag="kp", padded_shape=[P, 450]) for _ in range(8)]
            for cic in range(CIC):
                r32 = rp2.tile([P, NPIX], F32, name="r32", tag="r32")
                nc.gpsimd.dma_start(r32[:], f1gc[cic])
                nc.vector.tensor_copy(f1h[cic][:], r32[:])
                for nt in range(8):
                    nc.tensor.matmul(
                        kps[nt], wkt[cic][:], r32[:, nt * 450: (nt + 1) * 450],
                        start=cic == 0, stop=cic == CIC - 1,
                    )
            for nt in range(8):
                nc.vector.tensor_scalar_add(
                    ka[0:CQ, nt * 450: (nt + 1) * 450], kps[nt], bk[:]
                )

            # q from the f32 window
            for hf in range(2):
                qp = pk.tile([CQ, 512], F32, name="qp", tag="kp", padded_shape=[P, 512])
                for cic in range(CIC):
                    nc.tensor.matmul(
                        qp, wqt[cic][:], f1win[cic][:, hf * 512: (hf + 1) * 512],
                        start=cic == 0, stop=cic == CIC - 1,
                    )
                nc.vector.tensor_scalar_add(
                    qa[0:CQ, hf * 512: (hf + 1) * 512], qp, bq[:]
                )

            # v = wv @ f1 (bf16), then transpose
            for cot in range(CIC):
                vsb = vsp.tile([P, MP], BF16, name="vsb", tag="vsb")
                nc.any.memset(vsb[:, NPIX:MP], 0.0)
                for nt in range(8):
                    vp = pk.tile([P, 450], F32, name="vp", tag="kp")
                    for cic in range(CIC):
                        nc.tensor.matmul(
                            vp,
                            wvt[cic][:, cot * P: (cot + 1) * P],
                            f1h[cic][:, nt * 450: (nt + 1) * 450],
                            start=cic == 0, stop=cic == CIC - 1,
                        )
                    nc.vector.tensor_scalar_add(
                        vsb[:, nt * 450: (nt + 1) * 450], vp, bv[:, cot: cot + 1]
                    )
                nc.sync.dma_start_transpose(vt[cot][:], vsb[:])

        # ================= phase 4a: CAM gram matrix (overlaps AR) ===========
        xfwin = [pers.tile([P, WINP], BF16, name=f"xfwin{i}") for i in range(CIC)]
        cen_sb = [mid.tile([P, CI], F32, name=f"cen{i}") for i in range(CIC)]
        with ExitStack() as c4:
            sb4 = c4.enter_context(tc.tile_pool(name="sb4", bufs=1))
            pc = c4.enter_context(tc.tile_pool(name="pc", bufs=2, space="PSUM"))
            xfh = sb4.tile([P, CIC, WINP], BF16, name="xfh")
            xfl = sb4.tile([P, CIC, WINP], BF16, name="xfl")
            xth = sb4.tile([P, 8, CIC, P], BF16, name="xth")
            xtl = sb4.tile([P, 8, CIC, P], BF16, name="xtl")
            tmpf = sb4.tile([P, 900], F32, name="tmpf")
            for i in range(CIC):
                nc.any.memset(xfwin[i][:], 0.0)
                nc.vector.tensor_copy(xfwin[i][:, 0:WIN], f2win[i][:, 0:WIN])
                nc.any.memset(xfh[:, i, 900:WINP], 0.0)
                nc.any.memset(xfl[:, i, 900:WINP], 0.0)
                # hi/lo split of my 900 pixels (window cols 60:960)
                nc.vector.tensor_copy(xfh[:, i, 0:900], f2win[i][:, 60:960])
                nc.vector.tensor_copy(tmpf[:], xfh[:, i, 0:900])
                nc.vector.tensor_sub(xfl[:, i, 0:900], f2win[i][:, 60:960], tmpf[:])
                nc.sync.dma_start_transpose(xth[:, :, i, :], xfh[:, i, :])
                nc.sync.dma_start_transpose(xtl[:, :, i, :], xfl[:, i, :])
            for ct in range(CIC):
                cp = pc.tile([P, CI], F32, name="cp", tag="cp")
                n_mm = 0
                for nch in range(8):
                    for lh, rh in ((xth, xth), (xth, xtl), (xtl, xth)):
                        nc.tensor.matmul(
                            cp, lh[:, nch, ct, :], rh[:, nch, :, :].rearrange("p a b -> p (a b)"),
                            start=n_mm == 0, stop=n_mm == 23,
                        )
                        n_mm += 1
                nc.scalar.activation(cen_sb[ct][:], cp[:], AF.Copy)
                nc.sync.dma_start(cen_in[ct], cen_sb[ct][:])
            nc.gpsimd.collective_compute(
                "AllReduce", ALU.add,
                replica_groups=GROUPS,
                ins=[cen_in.opt()], outs=[cen_out.opt()],
            )

        # ======================= phase 3: position attention =================
        sa_win = [mid.tile([P, WINP], BF16, name=f"sawin{i}") for i in range(CIC)]
        with ExitStack() as c3:
            sb3 = c3.enter_context(tc.tile_pool(name="sb3", bufs=1))
            ap3 = c3.enter_context(tc.tile_pool(name="ap3", bufs=3))
            pe3 = c3.enter_context(tc.tile_pool(name="pe3", bufs=2, space="PSUM"))
            psa = c3.enter_context(tc.tile_pool(name="psa", bufs=4, space="PSUM"))
            psum3 = c3.enter_context(tc.tile_pool(name="psum3", bufs=2, space="PSUM"))

            ones = sb3.tile([P, 1], BF16, name="ones")
            nc.any.memset(ones[:], 1.0)
            nshift = sb3.tile([P, 1], F32, name="nshift")
            nc.any.memset(nshift[:], -CSH)
            for hf in range(2):
                hsl = slice(hf * 512, (hf + 1) * 512)
                saps = [
                    psa.tile([P, 512], F32, name="sap", tag="sap") for _ in range(CIC)
                ]
                sums = psum3.tile([1, 512], F32, name="sums", tag="sums", padded_shape=[P, 512])
                for mc in range(MCH):
                    ep = pe3.tile([P, 512], F32, name="ep", tag="ep")
                    nc.tensor.matmul(
                        ep, ka[:, mc * P: (mc + 1) * P], qa[:, hsl],
                        start=True, stop=True,
                    )
                    at = ap3.tile([P, 512], BF16, name="at", tag="at")
                    nc.scalar.activation(at[:], ep[:], AF.Exp, bias=nshift[:], scale=1.0)
                    nc.tensor.matmul(
                        sums, ones[:], at[:], start=mc == 0, stop=mc == MCH - 1
                    )
                    for cot in range(CIC):
                        nc.tensor.matmul(
                            saps[cot], vt[cot][:, mc, :], at[:],
                            start=mc == 0, stop=mc == MCH - 1,
                        )
                ssb = sb3.tile([1, 512], F32, name="ssb", tag="ssb", padded_shape=[P, 512])
                nc.scalar.activation(ssb[:], sums[:], AF.Copy)
                rec = sb3.tile([1, 512], F32, name="rec", tag="rec", padded_shape=[P, 512])
                nc.vector.reciprocal(rec[:], ssb[:])
                nc.vector.tensor_mul(rec[:], rec[:], qmask[:, hsl])
                rbp = pe3.tile([P, 512], F32, name="rbp", tag="ep")
                nc.tensor.matmul(rbp, gsa[:], rec[:], start=True, stop=True)
                recb = sb3.tile([P, 512], F32, name="recb", tag="recb")
                nc.scalar.activation(recb[:], rbp[:], AF.Copy)
                for cot in range(CIC):
                    tmp3 = sb3.tile([P, 512], F32, name="tmp3", tag="tmp3")
                    nc.vector.tensor_mul(tmp3[:], saps[cot][:], recb[:])
                    nc.vector.tensor_add(
                        sa_win[cot][:, hsl], tmp3[:], f1win[cot][:, hsl]
                    )

        # =================== phase 4b: CAM softmax + attention ===============
        sc_win = [mid.tile([P, WINP], BF16, name=f"scwin{i}") for i in range(CIC)]
        with ExitStack() as c4b:
            sb4b = c4b.enter_context(tc.tile_pool(name="sb4b", bufs=1))
            pc2 = c4b.enter_context(tc.tile_pool(name="pc2", bufs=2, space="PSUM"))
            cattT = sb4b.tile([P, CIC, CIC, P], BF16, name="cattT")
            crec = sb4b.tile([P, CIC], F32, name="crec")
            for ct in range(CIC):
                cg = cen_sb[ct]
                nc.sync.dma_start(cg[:], cen_out[ct])
                rmin = sb4b.tile([P, 1], F32, name="rmin", tag="rmin")
                nc.vector.tensor_reduce(rmin[:], cg[:], axis=AX.X, op=ALU.min)
                cat = sb4b.tile([P, CI], BF16, name="cat", tag="cat", bufs=2)
                csum = sb4b.tile([P, 1], F32, name="csum", tag="csum", bufs=2)
                nc.scalar.activation(
                    cat[:], cg[:], AF.Exp, bias=rmin[:], scale=-1.0,
                    accum_out=csum[:],
                )
                nc.vector.reciprocal(crec[:, ct: ct + 1], csum[:])
                nc.vector.tensor_mul(crec[:, ct: ct + 1], crec[:, ct: ct + 1], gsc[:])
                nc.sync.dma_start_transpose(cattT[:, :, ct, :], cat[:])
            for ct in range(CIC):
                for hf in range(2):
                    hsl = slice(hf * 512, (hf + 1) * 512)
                    scp = pc2.tile([P, 512], F32, name="scp", tag="scp")
                    for dch in range(CIC):
                        nc.tensor.matmul(
                            scp, cattT[:, dch, ct, :], xfwin[dch][:, hsl],
                            start=dch == 0, stop=dch == CIC - 1,
                        )
                    tmp4 = sb4b.tile([P, 512], F32, name="tmp4", tag="tmp4")
                    nc.scalar.activation(tmp4[:], scp[:], AF.Copy, scale=crec[:, ct: ct + 1])
                    nc.vector.tensor_add(
                        sc_win[ct][:, hsl], tmp4[:], f2win[ct][:, hsl]
                    )

        # ============= phase 5: pads, stage-2 convs, output heads ============
        late = ctx.enter_context(tc.tile_pool(name="late", bufs=1))
        sa_pad = [late.tile([P, 17, HP], BF16, name=f"sapad{i}") for i in range(CIC)]
        sc_pad = [late.tile([P, 17, HP], BF16, name=f"scpad{i}") for i in range(CIC)]
        for i in range(CIC):
            nc.any.memset(sa_pad[i][:], 0.0)
            nc.any.memset(sc_pad[i][:], 0.0)
            nc.vector.tensor_copy(
                sa_pad[i][:, :, 1:61],
                sa_win[i][:, 0:WIN].rearrange("p (r c) -> p r c", c=H),
            )
            nc.vector.tensor_copy(
                sc_pad[i][:, :, 1:61],
                sc_win[i][:, 0:WIN].rearrange("p (r c) -> p r c", c=H),
            )

        sa_conv = [late.tile([P, 900], BF16, name=f"sacv{i}") for i in range(CIC)]
        sc_conv = [late.tile([P, 900], BF16, name=f"sccv{i}") for i in range(CIC)]
        fsum = [late.tile([P, 900], BF16, name=f"fsum{i}") for i in range(CIC)]

        with ExitStack() as c5:
            sb5 = c5.enter_context(tc.tile_pool(name="sb5", bufs=1))
            wp5 = c5.enter_context(tc.tile_pool(name="wp5", bufs=4))
            pp5 = c5.enter_context(tc.tile_pool(name="pp5", bufs=3, space="PSUM"))
            ph5 = c5.enter_context(tc.tile_pool(name="ph5", bufs=2, space="PSUM"))

            bn1 = sb5.tile([P, 2, 2, CIC], F32, name="bn1")
            nc.sync.dma_start(bn1[:, 0], BN1S[:])
            nc.sync.dma_start(bn1[:, 1], BN1C[:])

            for bi, (wsrc, pad, cv) in enumerate(
                ((W1S, sa_pad, sa_conv), (W1C, sc_pad, sc_conv))
            ):
                for cot in range(CIC):
                    cps = [
                        pp5.tile([P, 300], F32, name="cp5", tag="cp5")
                        for _ in range(3)
                    ]
                    for cic in range(CIC):
                        for off in range(9):
                            ky, kx = off // 3, off % 3
                            wt = wp5.tile([P, P], BF16, name="w1t", tag="w1t")
                            nc.gpsimd.dma_start(
                                wt[:], wsrc[off, cic][:, cot * P: (cot + 1) * P]
                            )
                            start = cic == 0 and off == 0
                            stop = cic == CIC - 1 and off == 8
                            for rt in range(3):
                                rhs = pad[cic][
                                    :, rt * 5 + ky: rt * 5 + ky + 5, kx: kx + H
                                ]
                                nc.tensor.matmul(
                                    cps[rt], wt[:], rhs, start=start, stop=stop
                                )
                    for rt in range(3):
                        nc.scalar.activation(
                            cv[cot][:, rt * 300: (rt + 1) * 300], cps[rt][:],
                            AF.Relu, bias=bn1[:, bi, 1, cot: cot + 1], scale=bn1[:, bi, 0, cot: cot + 1],
                        )
            for i in range(CIC):
                nc.vector.tensor_add(fsum[i][:], sa_conv[i][:], sc_conv[i][:])

            w6 = sb5.tile([P, 3, CIC, CO], BF16, name="w6")
            b6 = sb5.tile([CO, 3], F32, name="b6", padded_shape=[P, 3])
            for j, (wsrc, bsrc) in enumerate(((W8T, B8), (W6T, B6), (W7T, B7))):
                for cic in range(CIC):
                    nc.sync.dma_start(w6[:, j, cic, :], wsrc[cic])
                nc.sync.dma_start(b6[:, j: j + 1], bsrc[:])
            for oi, src in enumerate((fsum, sa_conv, sc_conv)):
                for hf in range(2):
                    hp = ph5.tile([CO, 450], F32, name="hp", tag="hp", padded_shape=[P, 450])
                    for cic in range(CIC):
                        nc.tensor.matmul(
                            hp, w6[:, oi, cic, :], src[cic][:, hf * 450: (hf + 1) * 450],
                            start=cic == 0, stop=cic == CIC - 1,
                        )
                    osb = sb5.tile([CO, 450], F32, name="osb", tag="osb", padded_shape=[P, 450])
                    nc.vector.tensor_scalar_add(osb[:], hp[:], b6[:, oi: oi + 1])
                    nc.sync.dma_start(OUT[oi, :, hf * 450: (hf + 1) * 450], osb[:])

    if split:
        _split_waits(nc)
    return nc


# ------------------------------------------------------------- host side ---

def _bn_fold(p):
    s, b, m, v = np.asarray(p, np.float32)
    a = s / np.sqrt(v + EPS)
    return a.astype(np.float32), (b - m * a).astype(np.float32)


def host_prep(inputs):
    """Build the 8 per-core input maps."""
    inp = {k: np.asarray(v) for k, v in inputs.items()}
    x = inp["x"].astype(np.float32)
    d = inp["d"].astype(np.float32)
    lam = np.float32(inp["lamb"])
    B = x.shape[0]

    def conv_w_slice(w, s):
        # [O, I, 3, 3] -> lhsT layout [9, I//128, 128, 128] for O-slice s
        ws = w[s * P:(s + 1) * P]                       # [128, I, 3, 3]
        t = np.transpose(ws, (2, 3, 1, 0))              # [3, 3, I, 128]
        return np.ascontiguousarray(
            t.reshape(9, -1, P, P).astype(bf)
        )

    def conv_w_full(w):
        # [512, 512, 3, 3] -> [9, 4, 128, 512]
        t = np.transpose(w, (2, 3, 1, 0))               # [3,3,512,512]
        return np.ascontiguousarray(t.reshape(9, CIC, P, CI).astype(bf))

    xpads = []
    for b_ in range(B):
        xp = np.zeros((CIN, HP, HP), np.float32)
        xp[:, 1:61, 1:61] = x[b_]
        xpads.append(
            np.ascontiguousarray(xp.reshape(CICN, P, HP * HP).astype(bf))
        )

    a0s, b0s = _bn_fold(inp["bn_s0"])
    a0c, b0c = _bn_fold(inp["bn_c0"])
    a1s, b1s = _bn_fold(inp["bn_s1"])
    a1c, b1c = _bn_fold(inp["bn_c1"])

    wqt = np.ascontiguousarray(
        inp["wq"].T.reshape(CIC, P, CQ).astype(np.float32))
    wkt = np.ascontiguousarray(
        inp["wk"].T.reshape(CIC, P, CQ).astype(np.float32))
    wvt = np.ascontiguousarray(inp["wv"].T.reshape(CIC, P, CI).astype(bf))
    w1s = conv_w_full(inp["w_s1"])
    w1c = conv_w_full(inp["w_c1"])
    w6t = np.ascontiguousarray(inp["w6"].T.reshape(CIC, P, CO).astype(bf))
    w7t = np.ascontiguousarray(inp["w7"].T.reshape(CIC, P, CO).astype(bf))
    w8t = np.ascontiguousarray(inp["w8"].T.reshape(CIC, P, CO).astype(bf))

    def bn1_layout(a, b):
        # [P, 2, CIC]: [:, 0, c] = a-slice c, [:, 1, c] = b-slice c
        st = np.stack([a.reshape(CIC, P), b.reshape(CIC, P)])   # [2, CIC, P]
        return np.ascontiguousarray(st.transpose(2, 0, 1).astype(np.float32))

    bn1s = bn1_layout(a1s, b1s)
    bn1c = bn1_layout(a1c, b1c)

    gsa = np.full((1, P), np.float32(inp["gamma_sa"]), np.float32)
    gsc = np.full((P, 1), np.float32(inp["gamma_sc"]), np.float32)

    in_maps = []
    for c in range(8):
        b_, s = c // 4, c % 4
        df = d[b_, 0].reshape(NPIX)
        dka = np.zeros((2, MP), np.float32)
        dka[0, :NPIX] = lam * df * df
        dka[0, NPIX:] = -1000.0
        dka[1, :NPIX] = df

        out_r0 = 15 * s
        dqa = np.zeros((2, WINP), np.float32)
        qmask = np.zeros((1, WINP), np.float32)
        dqa[0, :WIN] = 1.0
        for v_ in range(17):
            rv = out_r0 - 1 + v_
            if 0 <= rv < H:
                dqa[1, v_ * H:(v_ + 1) * H] = -2.0 * lam * d[b_, 0, rv]
                qmask[0, v_ * H:(v_ + 1) * H] = 1.0

        in_maps.append({
            "XPAD": xpads[b_],
            "W0S": conv_w_slice(inp["w_s0"], s),
            "W0C": conv_w_slice(inp["w_c0"], s),
            "BN0S": np.stack([a0s[s * P:(s + 1) * P],
                              b0s[s * P:(s + 1) * P]], axis=1),
            "BN0C": np.stack([a0c[s * P:(s + 1) * P],
                              b0c[s * P:(s + 1) * P]], axis=1),
            "WQT": wqt, "WKT": wkt,
            "BQ": inp["bq"].reshape(CQ, 1).astype(np.float32),
            "BK": inp["bk"].reshape(CQ, 1).astype(np.float32),
            "WVT": wvt,
            "BV": np.ascontiguousarray(
                inp["bv"].reshape(CIC, P).T.astype(np.float32)),
            "DKA": dka, "DQA": dqa, "QMASK": qmask,
            "GSA": gsa, "GSC": gsc,
            "W1S": w1s, "W1C": w1c, "BN1S": bn1s, "BN1C": bn1c,
            "W6T": w6t, "W7T": w7t, "W8T": w8t,
            "B6": inp["b6"].reshape(CO, 1).astype(np.float32),
            "B7": inp["b7"].reshape(CO, 1).astype(np.float32),
            "B8": inp["b8"].reshape(CO, 1).astype(np.float32),
            "ZPAD": np.zeros((CIC, P, H), np.float32),
        })
    return in_maps


def assemble(results):
    """results: list of 8 dicts with 'OUT' [3, 40, 900] -> output tuple."""
    outs = []
    for b_ in range(2):
        rows = [np.asarray(results[4 * b_ + s]["OUT"], np.float32).reshape(
            3, CO, 15, H) for s in range(4)]
        outs.append(np.concatenate(rows, axis=2))        # [3, 40, 60, 60]
    full = np.stack(outs, axis=1)                        # [3, B, 40, 60, 60]
    return full[0], full[1], full[2]


def _split_waits(nc, keep=1):
    """Walrus in this container accepts at most one embedded sync-wait per
    instruction; Tile emits several. Turn extra waits into standalone
    single-wait EventSemaphore instructions before the owner, same engine."""
    n_split = 0
    for fn in nc.m.functions:
        for bb in fn.blocks:
            new_insts = []
            for inst in bb.instructions:
                si = inst.sync_info
                if si is not None and len(si.on_wait) > keep:
                    waits = list(si.on_wait)
                    head, tail = waits[:-keep], waits[-keep:]
                    for j, w in enumerate(head):
                        new_insts.append(mybir.InstEventSemaphore(
                            name=f"{inst.name}-ws{j}",
                            engine=inst.engine,
                            ins=[], outs=[],
                            sync_info=mybir.SyncInfo(on_wait=[w], on_update=[]),
                        ))
                        n_split += 1
                    inst.sync_info = mybir.SyncInfo(
                        on_wait=tail, on_update=list(si.on_update))
                new_insts.append(inst)
            bb.instructions.clear()
            bb.instructions.extend(new_insts)
    return n_split


_NC = None


def kernel(**inputs):
    global _NC
    if _NC is None:
        _NC = build_nc()
    from concourse.bass_utils import run_bass_kernel_spmd
    in_maps = host_prep(inputs)
    res = run_bass_kernel_spmd(_NC, in_maps, core_ids=list(range(8)))
    return assemble(res.results)
